# revision 17
# baseline (speedup 1.0000x reference)
"""ChebNet (K=2, 4 layers, 128-dim) + mean-pool + MLP on 8 Trainium2 cores.

Strategy (graph/data parallel by destination-node blocks):
  - Nodes are split into 8 contiguous blocks (6250 each). Core c owns block c
    and all edges whose *destination* is in block c.
  - Per layer: every core computes hw1 = h_blk @ W1 for its block,
    AllGather -> full hw1 table (node-major, padded to 6272 rows/block) in
    DRAM on every core.
  - Edge aggregation per 128-node destination window via PSUM-accumulated
    matmuls: gather 128 source rows of hw1 (one indirect DMA per 128-edge
    tile), build a norm-scaled selection matrix S[e,d] = norm_e * [dest_e==d]
    on DVE, and accumulate G^T @ S into PSUM [feat, dest] — together with the
    dense part W0^T @ h^T and a K=1 bias matmul. ReLU writes the
    feature-major h for the next layer directly (no transpose needed).
  - Layer 4: additionally transpose each window and matmul with a one-hot
    graph-assignment matrix P to get partial pooled sums; AllReduce [64,128],
    scale by 1/count, then the tiny MLP head runs replicated on every core.

Self-contained: hardcodes the problem shapes; host-side prep only reorganizes
index/weight data (degree counts, edge->tile packing, one-hot pool matrix).
"""
import numpy as np
import jax
from jax.sharding import Mesh, PartitionSpec
from jax.experimental.shard_map import shard_map

from concourse import bass, bacc, tile, mybir, bass2jax
from concourse.bass2jax import _bass_exec_p, partition_id_tensor
from concourse.masks import make_identity

NCORES = 8
D = 128
NCONV = 4
NG = 64
P = 128

# tunables
GATHER_SPLIT = 1      # indirect DMAs per 128-edge tile (1, 2 or 4)
G_BUFS = 8            # gather tile double-buffering depth
S_BUFS = 6
SKIP_AG = False       # timing-attribution switches (break correctness)
SKIP_GATHER = False
SKIP_S = False
SKIP_EMM = False

F32 = mybir.dt.float32
I32 = mybir.dt.int32
DT_MSG = mybir.dt.float16   # dtype of the h@W1 message table / AG / S


# --------------------------------------------------------------------------
# host-side data prep
# --------------------------------------------------------------------------

def _host_prep(x, conv_W, conv_b, lin0_W, lin0_b, lin1_W, lin1_b,
               edge_index, batch):
    n, d = x.shape
    assert d == D and n % NCORES == 0
    blk = n // NCORES
    nw = (blk + P - 1) // P
    sh_pad = nw * P

    row = np.asarray(edge_index[0], np.int64)
    col = np.asarray(edge_index[1], np.int64)
    batch = np.asarray(batch, np.int64)

    deg = np.bincount(row, minlength=n).astype(np.float32)
    dis = np.where(deg > 0, 1.0 / np.sqrt(np.maximum(deg, 1.0)),
                   0.0).astype(np.float32)
    norm = -(dis[row] * dis[col]).astype(np.float32)

    core = row // blk
    lrow = row % blk
    win = lrow // P
    dloc = (lrow % P).astype(np.float32)

    # source-chunk split: chunk A = first nw_a windows' rows of each block
    nw_a = (nw + 1) // 2
    rows_a = nw_a * P
    rows_b = sh_pad - rows_a
    lcol = col % blk
    src_chunk = (lcol >= rows_a).astype(np.int64)
    rows_c = np.where(src_chunk == 0, rows_a, rows_b)
    col_remap = ((col // blk) * rows_c + (lcol - src_chunk * rows_a)
                 ).astype(np.int32)

    order = np.lexsort((col, src_chunk, win, core))
    core_s, win_s = core[order], win[order]
    chunk_s = src_chunk[order]
    colr_s, dloc_s, norm_s = col_remap[order], dloc[order], norm[order]

    cnt = np.zeros((NCORES, nw, 2), np.int64)
    np.add.at(cnt, (core, win, src_chunk), 1)
    tiles2 = ((cnt + P - 1) // P).max(axis=0)  # [nw, 2]
    # flat tile stream: chunk-major, then window
    tiles_flat = np.concatenate([tiles2[:, 0], tiles2[:, 1]])
    offs_flat = np.concatenate([[0], np.cumsum(tiles_flat)]).astype(np.int64)
    tt = int(offs_flat[-1])
    # offs[(c, w)] lookup
    offs2 = np.zeros((2, nw), np.int64)
    offs2[0, :] = offs_flat[:nw]
    offs2[1, :] = offs_flat[nw:2 * nw]

    seg_starts = np.zeros((NCORES, nw, 2), np.int64)
    flat_cnt = cnt.transpose(0, 1, 2).reshape(-1)
    seg_starts.reshape(-1)[1:] = np.cumsum(flat_cnt)[:-1]

    col_arr = np.zeros((NCORES, P, tt), np.int32)
    dest_arr = np.zeros((NCORES, P, tt), np.float32)
    norm_arr = np.zeros((NCORES, P, tt), np.float32)
    for c in range(NCORES):
        for w in range(nw):
            for h in range(2):
                k = cnt[c, w, h]
                if k == 0:
                    continue
                s0 = seg_starts[c, w, h]
                j = np.arange(k)
                pp = j % P
                ti = offs2[h, w] + j // P
                col_arr[c, pp, ti] = colr_s[s0:s0 + k]
                dest_arr[c, pp, ti] = dloc_s[s0:s0 + k]
                norm_arr[c, pp, ti] = norm_s[s0:s0 + k]

    # feature-major x blocks, zero-padded to sh_pad columns
    x = np.asarray(x, np.float32)
    xfm = np.zeros((NCORES, P, sh_pad), np.float32)
    for c in range(NCORES):
        xfm[c, :, :blk] = x[c * blk:(c + 1) * blk].T

    # pooling one-hot [P, nw*NG] per core; count reciprocal
    counts = np.bincount(batch, minlength=NG).astype(np.float32)
    recip = (1.0 / np.maximum(counts, 1.0)).astype(np.float32)[:, None]
    pall = np.zeros((NCORES, P, nw * NG), np.float32)
    for c in range(NCORES):
        g_blk = batch[c * blk:(c + 1) * blk]
        l = np.arange(blk)
        pall[c, l % P, (l // P) * NG + g_blk] = 1.0

    wall = np.asarray(conv_W, np.float32).transpose(2, 0, 1, 3).reshape(
        P, NCONV * 2 * D)                       # [fi, (2i+k)*128+fo]
    biasr = np.asarray(conv_b, np.float32).reshape(1, NCONV * D)
    jmat = np.tile(np.arange(P, dtype=np.float32), (P, 1))

    common = {
        "wall": wall, "biasr": biasr, "jmat": jmat,
        "l0w": np.asarray(lin0_W, np.float32),
        "l0b": np.asarray(lin0_b, np.float32).reshape(-1, 1),
        "l1w": np.asarray(lin1_W, np.float32),
        "l1b": np.asarray(lin1_b, np.float32).reshape(-1, 1),
        "recip": recip,
    }
    in_maps = []
    for c in range(NCORES):
        m = dict(common)
        m["xfm"] = xfm[c]
        m["colr"] = col_arr[c]
        for sp in (2, 4):
            npp = P // sp
            for s in range(sp):
                m[f"colr{sp}_{s}"] = np.ascontiguousarray(
                    col_arr[c, s * npp:(s + 1) * npp, :])
        m["destn"] = dest_arr[c]
        m["normv"] = norm_arr[c]
        m["pall"] = pall[c]
        in_maps.append(m)

    meta = dict(n=n, blk=blk, nw=nw, sh_pad=sh_pad, tt=tt, nw_a=nw_a,
                rows_a=rows_a, rows_b=rows_b,
                tiles=tuple(tuple(int(t) for t in tiles2[:, h])
                            for h in range(2)),
                offs=tuple(tuple(int(o) for o in offs2[h])
                           for h in range(2)))
    return in_maps, meta


# --------------------------------------------------------------------------
# device program
# --------------------------------------------------------------------------

def _build_program(meta, repeat=1):
    nw, sh_pad, tt = meta["nw"], meta["sh_pad"], meta["tt"]
    tiles, offs = meta["tiles"], meta["offs"]
    tmax = max(max(tiles[0]), max(tiles[1]), 1)
    n_pad = NCORES * sh_pad

    nc = bacc.Bacc("TRN2", target_bir_lowering=False, debug=False,
                   num_devices=NCORES)

    xfm_in = nc.dram_tensor("xfm", [P, sh_pad], F32, kind="ExternalInput")
    if GATHER_SPLIT == 1:
        colr_ins = [nc.dram_tensor("colr", [P, tt], I32,
                                   kind="ExternalInput")]
    else:
        npp = P // GATHER_SPLIT
        colr_ins = [nc.dram_tensor(f"colr{GATHER_SPLIT}_{s}", [npp, tt], I32,
                                   kind="ExternalInput")
                    for s in range(GATHER_SPLIT)]
    destn_in = nc.dram_tensor("destn", [P, tt], F32, kind="ExternalInput")
    normv_in = nc.dram_tensor("normv", [P, tt], F32, kind="ExternalInput")
    wall_in = nc.dram_tensor("wall", [P, NCONV * 2 * D], F32,
                             kind="ExternalInput")
    biasr_in = nc.dram_tensor("biasr", [1, NCONV * D], F32,
                              kind="ExternalInput")
    jmat_in = nc.dram_tensor("jmat", [P, P], F32, kind="ExternalInput")
    pall_in = nc.dram_tensor("pall", [P, nw * NG], F32, kind="ExternalInput")
    recip_in = nc.dram_tensor("recip", [NG, 1], F32, kind="ExternalInput")
    l0w_in = nc.dram_tensor("l0w", [D, D // 2], F32, kind="ExternalInput")
    l0b_in = nc.dram_tensor("l0b", [D // 2, 1], F32, kind="ExternalInput")
    l1w_in = nc.dram_tensor("l1w", [D // 2, 10], F32, kind="ExternalInput")
    l1b_in = nc.dram_tensor("l1b", [10, 1], F32, kind="ExternalInput")
    out_t = nc.dram_tensor("outt", [10, NG], F32, kind="ExternalOutput")

    with tile.TileContext(nc) as tc:
        with (
            tc.tile_pool(name="sb", bufs=1) as sb,
            tc.tile_pool(name="gpool", bufs=G_BUFS) as gpool,
            tc.tile_pool(name="spool", bufs=S_BUFS) as spool,
            tc.tile_pool(name="hpool", bufs=2) as hpool,
            tc.tile_pool(name="ppsum", bufs=2, space="PSUM") as ppsum,
            tc.tile_pool(name="dram", bufs=1, space="DRAM") as dram,
        ):
            # ---------------- preload ----------------
            h_a = sb.tile([P, sh_pad], F32)
            h_b = sb.tile([P, sh_pad], F32)
            colr_sbs = []
            for s in range(GATHER_SPLIT):
                npp = P // GATHER_SPLIT
                csb = sb.tile([npp, tt], I32, name=f"colrsb{s}",
                              tag=f"colrsb{s}")
                nc.sync.dma_start(out=csb[:], in_=colr_ins[s][:])
                colr_sbs.append(csb)
            destn_sb = sb.tile([P, tt], F32)
            nc.sync.dma_start(out=destn_sb[:], in_=destn_in[:])
            normv_sb = sb.tile([P, tt], F32)
            nc.sync.dma_start(out=normv_sb[:], in_=normv_in[:])
            wall_sb = sb.tile([P, NCONV * 2 * D], F32)
            nc.sync.dma_start(out=wall_sb[:], in_=wall_in[:])
            biasr_sb = sb.tile([1, NCONV * D], F32)
            nc.sync.dma_start(out=biasr_sb[:], in_=biasr_in[:])
            jmat_sb = sb.tile([P, P], F32)
            nc.sync.dma_start(out=jmat_sb[:], in_=jmat_in[:])
            pall_sb = sb.tile([P, nw * NG], F32)
            nc.sync.dma_start(out=pall_sb[:], in_=pall_in[:])
            recip_sb = sb.tile([NG, 1], F32)
            nc.sync.dma_start(out=recip_sb[:], in_=recip_in[:])
            l0w_sb = sb.tile([D, D // 2], F32)
            nc.sync.dma_start(out=l0w_sb[:], in_=l0w_in[:])
            l0b_sb = sb.tile([D // 2, 1], F32)
            nc.sync.dma_start(out=l0b_sb[:], in_=l0b_in[:])
            l1w_sb = sb.tile([D // 2, 10], F32)
            nc.sync.dma_start(out=l1w_sb[:], in_=l1w_in[:])
            l1b_sb = sb.tile([10, 1], F32)
            nc.sync.dma_start(out=l1b_sb[:], in_=l1b_in[:])

            ones_sb = sb.tile([1, P], F32)
            nc.vector.memset(ones_sb[:], 1.0)
            ident = sb.tile([P, P], F32)
            make_identity(nc, ident[:])
            pool_acc = sb.tile([NG, D], F32)
            h_acc = sb.tile([P, sh_pad], F32)

            hw1_stage = sb.tile([P, sh_pad], DT_MSG)

            for rep in range(repeat):
                _one_pass(nc, tc, sb, gpool, spool, hpool, ppsum, dram, rep,
                          meta, tmax, n_pad, h_a, h_b, xfm_in, colr_sbs,
                          destn_sb, normv_sb, wall_sb, biasr_sb, jmat_sb,
                          pall_sb, recip_sb, l0w_sb, l0b_sb, l1w_sb, l1b_sb,
                          ones_sb, ident, pool_acc, h_acc, hw1_stage,
                          out_t)

    nc.compile()
    return nc


def _one_pass(nc, tc, sb, gpool, spool, hpool, ppsum, dram, rep, meta,
              tmax, n_pad, h_a, h_b, xfm_in, colr_sbs, destn_sb, normv_sb,
              wall_sb, biasr_sb, jmat_sb, pall_sb, recip_sb, l0w_sb, l0b_sb,
              l1w_sb, l1b_sb, ones_sb, ident, pool_acc, h_acc, hw1_stage,
              out_t):
    nw, sh_pad = meta["nw"], meta["sh_pad"]
    tiles, offs = meta["tiles"], meta["offs"]
    if True:
        if True:
            nc.sync.dma_start(out=h_a[:], in_=xfm_in[:])
            nc.vector.memset(pool_acc[:], 0.0)
            nw_a, rows_a, rows_b = meta["nw_a"], meta["rows_a"], meta["rows_b"]
            rows_h = (rows_a, rows_b)
            hw1_shards = [[dram.tile([rows_h[h], D], DT_MSG,
                                     name=f"hw1s{rep}_{i}_{h}")
                           for h in range(2)] for i in range(NCONV)]
            hw1_fulls = [[dram.tile([NCORES * rows_h[h], D], DT_MSG,
                                    addr_space="Shared",
                                    name=f"hw1f{rep}_{i}_{h}")
                          for h in range(2)] for i in range(NCONV)]
            pool_in_d = dram.tile([NG, D], F32, name=f"pli{rep}")
            pool_out_d = dram.tile([NG, D], F32, addr_space="Shared",
                                   name=f"plo{rep}")

            h_cur, h_nxt = h_a, h_b
            for i in range(NCONV):
                w0 = wall_sb[:, (2 * i) * D:(2 * i + 1) * D]
                w1 = wall_sb[:, (2 * i + 1) * D:(2 * i + 2) * D]
                bias_i = biasr_sb[:, i * D:(i + 1) * D]

                # ---- hw1 = h @ W1 for own block; stage + AllGather ----
                for w in range(nw):
                    ph = ppsum.tile([P, D], F32, tag="ph", name=f"ph{rep}_{i}_{w}")
                    nc.tensor.matmul(ph[:],
                                     lhsT=h_cur[:, w * P:(w + 1) * P],
                                     rhs=w1, start=True, stop=True)
                    nc.scalar.activation(hw1_stage[:, w * D:(w + 1) * D],
                                         ph[:],
                                         mybir.ActivationFunctionType.Copy)
                nc.sync.dma_start(
                    out=hw1_shards[i][0][:].rearrange("(w p) f -> p w f",
                                                      p=P),
                    in_=hw1_stage[:, :nw_a * D].rearrange(
                        "p (w f) -> p w f", f=D))
                nc.sync.dma_start(
                    out=hw1_shards[i][1][:].rearrange("(w p) f -> p w f",
                                                      p=P),
                    in_=hw1_stage[:, nw_a * D:].rearrange(
                        "p (w f) -> p w f", f=D))
                for h in range(2):
                    nc.gpsimd.collective_compute(
                        "AllGather", mybir.AluOpType.bypass,
                        replica_groups=[list(range(NCORES))],
                        ins=[hw1_shards[i][h][:].opt()],
                        outs=[hw1_fulls[i][h][:].opt()])

                # ---- window loop, source-chunk-major so chunk B's
                # AllGather transfer overlaps chunk A's gather work ----
                for h in range(2):
                    for w in range(nw):
                        tw = tiles[h][w]
                        off = offs[h][w]
                        wc = slice(w * P, (w + 1) * P)
                        pa = None
                        if h == 0 or tw > 0:
                            pa = ppsum.tile([P, P], F32, tag="pa",
                                            name=f"pa{rep}_{i}_{h}_{w}")
                        if h == 0:
                            nc.tensor.matmul(pa[:], lhsT=w0,
                                             rhs=h_cur[:, wc],
                                             start=True, stop=False)
                            nc.tensor.matmul(pa[:], lhsT=bias_i,
                                             rhs=ones_sb[:],
                                             start=False, stop=(tw == 0))
                        if tw > 0:
                            gw = gpool.tile([P, tmax * D], DT_MSG, tag="gw",
                                            name=f"gw{rep}_{i}_{h}_{w}")
                            for t in range(tw):
                                nc.gpsimd.indirect_dma_start(
                                    out=gw[:, t * D:(t + 1) * D],
                                    out_offset=None,
                                    in_=hw1_fulls[i][h][:],
                                    in_offset=bass.IndirectOffsetOnAxis(
                                        ap=colr_sbs[0][:,
                                                      off + t:off + t + 1],
                                        axis=0))
                            sw = spool.tile([P, tmax * P], DT_MSG, tag="sw",
                                            name=f"sw{rep}_{i}_{h}_{w}")
                            sw3 = sw[:, :tw * P].rearrange(
                                "p (t f) -> p t f", f=P)
                            jb = jmat_sb[:, None, :].to_broadcast([P, tw, P])
                            db = destn_sb[:, off:off + tw].to_broadcast(
                                [P, tw, P])
                            nb = normv_sb[:, off:off + tw].to_broadcast(
                                [P, tw, P])
                            nc.vector.tensor_tensor(
                                out=sw3, in0=jb, in1=db,
                                op=mybir.AluOpType.is_equal)
                            nc.vector.tensor_tensor(
                                out=sw3, in0=sw3, in1=nb,
                                op=mybir.AluOpType.mult)
                            for t in range(tw):
                                nc.tensor.matmul(
                                    pa[:],
                                    lhsT=gw[:, t * D:(t + 1) * D],
                                    rhs=sw[:, t * P:(t + 1) * P],
                                    start=(h != 0 and t == 0),
                                    stop=(t == tw - 1))
                        if h == 0:
                            nc.vector.tensor_copy(h_acc[:, wc], pa[:])
                        else:
                            if tw > 0:
                                nc.vector.tensor_add(h_acc[:, wc],
                                                     h_acc[:, wc], pa[:])
                            nc.scalar.activation(
                                h_nxt[:, wc], h_acc[:, wc],
                                mybir.ActivationFunctionType.Relu)
                            if i == NCONV - 1:
                                pt = ppsum.tile([P, P], F32, tag="pt",
                                                name=f"pt{rep}_{w}")
                                nc.tensor.transpose(pt[:], h_nxt[:, wc],
                                                    ident[:])
                                hnm = hpool.tile([P, P], F32, tag="hnm",
                                                 name=f"hnm{rep}_{w}")
                                nc.scalar.activation(
                                    hnm[:], pt[:],
                                    mybir.ActivationFunctionType.Copy)
                                pp = ppsum.tile([NG, D], F32, tag="pp",
                                                name=f"pp{rep}_{w}")
                                nc.tensor.matmul(
                                    pp[:],
                                    lhsT=pall_sb[:, w * NG:(w + 1) * NG],
                                    rhs=hnm[:], start=True, stop=True)
                                nc.vector.tensor_add(pool_acc[:],
                                                     pool_acc[:], pp[:])
                h_cur, h_nxt = h_nxt, h_cur

            # ---- pooling AllReduce + MLP head (replicated) ----
            nc.sync.dma_start(out=pool_in_d[:], in_=pool_acc[:])
            nc.gpsimd.collective_compute(
                "AllReduce", mybir.AluOpType.add,
                replica_groups=[list(range(NCORES))],
                ins=[pool_in_d[:].opt()], outs=[pool_out_d[:].opt()])
            pooled_sb = sb.tile([NG, D], F32, name=f"pooled{rep}", tag="pooled")
            nc.sync.dma_start(out=pooled_sb[:], in_=pool_out_d[:])
            nc.vector.tensor_scalar_mul(pooled_sb[:], pooled_sb[:],
                                        recip_sb[:, :1])
            ptr = ppsum.tile([D, NG], F32, tag="pt", name=f"ptr{rep}")
            nc.tensor.transpose(ptr[:], pooled_sb[:], ident[:NG, :NG])
            pooledT = sb.tile([D, NG], F32, name=f"pooledT{rep}", tag="pooledT")
            nc.scalar.activation(pooledT[:], ptr[:],
                                 mybir.ActivationFunctionType.Copy)
            pz = ppsum.tile([D // 2, NG], F32, tag="pp", name=f"pz{rep}")
            nc.tensor.matmul(pz[:], lhsT=l0w_sb[:], rhs=pooledT[:],
                             start=True, stop=True)
            z_sb = sb.tile([D // 2, NG], F32, name=f"zsb{rep}", tag="zsb")
            nc.scalar.activation(z_sb[:], pz[:],
                                 mybir.ActivationFunctionType.Relu,
                                 bias=l0b_sb[:, :1])
            po = ppsum.tile([10, NG], F32, tag="pp", name=f"po{rep}")
            nc.tensor.matmul(po[:], lhsT=l1w_sb[:], rhs=z_sb[:],
                             start=True, stop=True)
            out_sb = sb.tile([10, NG], F32, name=f"osb{rep}", tag="osb")
            nc.vector.tensor_scalar(out=out_sb[:], in0=po[:],
                                    scalar1=l1b_sb[:, :1], scalar2=None,
                                    op0=mybir.AluOpType.add)
            nc.sync.dma_start(out=out_t[:], in_=out_sb[:])

    nc.compile()
    return nc


# --------------------------------------------------------------------------
# PJRT runner (jit once, reuse across calls)
# --------------------------------------------------------------------------

class _SpmdRunner:
    def __init__(self, nc, n_cores):
        bass2jax.install_neuronx_cc_hook()
        self.nc = nc
        self.n_cores = n_cores
        pname = nc.partition_id_tensor.name if nc.partition_id_tensor else None
        in_names, out_names, out_avals = [], [], []
        for alloc in nc.m.functions[0].allocations:
            if not isinstance(alloc, mybir.MemoryLocationSet):
                continue
            name = alloc.memorylocations[0].name
            if alloc.kind == "ExternalInput":
                if name != pname:
                    in_names.append(name)
            elif alloc.kind == "ExternalOutput":
                out_names.append(name)
                out_avals.append(jax.core.ShapedArray(
                    tuple(alloc.tensor_shape), mybir.dt.np(alloc.dtype)))
        self.in_names, self.out_names, self.out_avals = (
            in_names, out_names, out_avals)
        all_in = list(in_names) + list(out_names)
        if pname is not None:
            all_in.append(pname)

        def _body(*args):
            operands = list(args)
            if pname is not None:
                operands.append(partition_id_tensor())
            return tuple(_bass_exec_p.bind(
                *operands, out_avals=tuple(out_avals),
                in_names=tuple(all_in), out_names=tuple(out_names),
                lowering_input_output_aliases=(),
                sim_require_finite=True, sim_require_nnan=True, nc=nc))

        devices = jax.devices()[:n_cores]
        self.mesh = Mesh(np.asarray(devices), ("core",))
        n_args = len(in_names) + len(out_names)
        self.fn = jax.jit(
            shard_map(_body, mesh=self.mesh,
                      in_specs=(PartitionSpec("core"),) * n_args,
                      out_specs=(PartitionSpec("core"),) * len(out_names),
                      check_rep=False),
            keep_unused=True)

    def put_inputs(self, in_maps):
        sh = jax.sharding.NamedSharding(self.mesh, PartitionSpec("core"))
        return [jax.device_put(
            np.concatenate([np.asarray(m[n]) for m in in_maps], axis=0), sh)
            for n in self.in_names]

    def run(self, dev_inputs):
        sh = jax.sharding.NamedSharding(self.mesh, PartitionSpec("core"))
        zeros = [jax.device_put(
            np.zeros((self.n_cores * a.shape[0], *a.shape[1:]), a.dtype), sh)
            for a in self.out_avals]
        outs = self.fn(*dev_inputs, *zeros)
        jax.block_until_ready(outs)
        return outs

    def results(self, outs):
        return [{n: np.asarray(outs[i]).reshape(
            self.n_cores, *self.out_avals[i].shape)[c]
            for i, n in enumerate(self.out_names)}
            for c in range(self.n_cores)]


_CACHE = {}


def _get_runner(meta):
    key = (meta["n"], meta["tt"], meta["tiles"], GATHER_SPLIT)
    if key not in _CACHE:
        nc = _build_program(meta)
        _CACHE[key] = _SpmdRunner(nc, NCORES)
    return _CACHE[key]


def kernel(**inputs):
    in_maps, meta = _host_prep(
        inputs["x"], inputs["conv_W"], inputs["conv_b"],
        inputs["lin0_W"], inputs["lin0_b"], inputs["lin1_W"],
        inputs["lin1_b"], inputs["edge_index"], inputs["batch"])
    runner = _get_runner(meta)
    din = runner.put_inputs(in_maps)
    outs = runner.run(din)
    res = runner.results(outs)
    return np.ascontiguousarray(res[0]["outt"].T)


# --------------------------------------------------------------------------
# selftest on a small random graph (numpy reference)
# --------------------------------------------------------------------------

def _np_reference(x, conv_W, conv_b, lin0_W, lin0_b, lin1_W, lin1_b,
                  edge_index, batch, n_graphs=NG):
    n = x.shape[0]
    row, col = edge_index[0], edge_index[1]
    deg = np.bincount(row, minlength=n).astype(np.float32)
    dis = np.where(deg > 0, 1.0 / np.sqrt(np.maximum(deg, 1.0)), 0.0)
    norm = -(dis[row] * dis[col]).astype(np.float32)
    h = x.astype(np.float32)
    for i in range(NCONV):
        tx1 = np.zeros_like(h)
        np.add.at(tx1, row, norm[:, None] * h[col])
        h = np.maximum(h @ conv_W[i, 0] + tx1 @ conv_W[i, 1] + conv_b[i], 0)
    cntg = np.bincount(batch, minlength=n_graphs).astype(np.float32)
    pooled = np.zeros((n_graphs, h.shape[1]), np.float32)
    np.add.at(pooled, batch, h)
    pooled /= np.maximum(cntg, 1.0)[:, None]
    z = np.maximum(pooled @ lin0_W + lin0_b, 0)
    return z @ lin1_W + lin1_b


def _selftest(n=2048, e=16384, seed=0):
    rng = np.random.default_rng(seed)
    inputs = dict(
        x=rng.standard_normal((n, D)).astype(np.float32),
        conv_W=(0.05 * rng.standard_normal((NCONV, 2, D, D))).astype(
            np.float32),
        conv_b=(0.01 * rng.standard_normal((NCONV, D))).astype(np.float32),
        lin0_W=(0.05 * rng.standard_normal((D, D // 2))).astype(np.float32),
        lin0_b=(0.01 * rng.standard_normal((D // 2,))).astype(np.float32),
        lin1_W=(0.05 * rng.standard_normal((D // 2, 10))).astype(np.float32),
        lin1_b=(0.01 * rng.standard_normal((10,))).astype(np.float32),
        edge_index=rng.integers(0, n, size=(2, e)).astype(np.int32),
        batch=np.sort(rng.integers(0, NG, size=(n,))).astype(np.int32),
    )
    exp = _np_reference(**inputs)
    act = kernel(**inputs)
    err = np.abs(act - exp).max() / max(np.abs(exp).max(), 1e-9)
    print(f"selftest n={n} e={e}: rel_err={err:.3e} "
          f"({'PASS' if err < 1e-4 else 'FAIL'})")
    return err


if __name__ == "__main__":
    _selftest()
